# revision 34
# baseline (speedup 1.0000x reference)
"""NT-Xent loss kernel for Trainium2, distributed across 8 NeuronCores.

Strategy: each core receives the full [8192, 128] input, rotated by 1024*c
rows so the kernel is pure SPMD — every core computes the row sums of
exp(sim/T) for the *first* 1024 local rows against all 8192 columns.

Per core:
  - load x (rows-on-partitions layout, 64 chunks of [128, 128])
  - norms:  s = sum(x^2, axis=d)  (gpsimd square + DVE reduce, per group)
  - r = s^(-1/2) via exp(-0.5 * ln(s))  (keeps ACT in the exp/ln table set)
  - normalize rows -> bf16 (gpsimd; DVE's instruction struct has too few
    sync-wait slots for DMA-queue deps, the Pool engine tolerates them)
  - PE-transpose chunks -> xnT [128(d), 8192(rows)] in 4 group tiles
  - main loop: 8 Mtiles x 4 col-groups: 4 bf16 matmuls -> PSUM [128,2048],
    one ACT pass exp(2*sim) with accum_out giving partial row sums
  - row totals - e^2 (diag), ln + accumulate, minus 2*sum(pos-pair sims),
    partition-reduce via ones-matmul -> scalar partial loss
Host sums the 8 partial scalars.

Hardware constraint honored throughout: a Matmult instruction can carry
only ONE sync-wait, so every matmul's dependencies (operands + PSUM slot
release) are arranged to come from a single semaphore (usually ACT's).
"""

import numpy as np

import concourse.bass as bass
import concourse.tile as tile
from concourse import mybir
from concourse.bass_utils import run_bass_kernel_spmd
from concourse.masks import make_identity

N2 = 8192          # total rows (2N)
D = 128            # feature dim
NCORES = 8
RPC = N2 // NCORES  # rows per core = 1024
NCHUNK = N2 // 128  # 64 chunks of 128 rows
F32 = mybir.dt.float32
BF16 = mybir.dt.bfloat16
AF = mybir.ActivationFunctionType
ALU = mybir.AluOpType
E2 = float(np.exp(2.0, dtype=np.float64))  # diag term exp(sim_ii / T), T=0.5


def _emit(tc: tile.TileContext, ctx, out_ap: bass.AP, x_ap: bass.AP):
    nc = tc.nc

    big = ctx.enter_context(tc.tile_pool(name="big", bufs=1))
    esc = ctx.enter_context(tc.tile_pool(name="esc", bufs=3))
    small = ctx.enter_context(tc.tile_pool(name="small", bufs=1))

    # one tile per DMA group: keeps each consumer waiting on a single DMA sem
    x_g = [
        big.tile([128, 8, 128], F32, tag=f"x{g}", name=f"x_{g}") for g in range(8)
    ]
    xsq_g = [
        big.tile([128, 8, 128], F32, tag=f"xsq{g}", name=f"xsq_{g}")
        for g in range(8)
    ]
    xb = big.tile([128, NCHUNK, 128], BF16, tag="xb")     # normalized, bf16
    # transposed normalized matrix, split into 4 tiles (finer matmul deps)
    xnT = [
        big.tile([128, 2048], BF16, tag=f"xnT{t}", name=f"xnT_{t}")
        for t in range(4)
    ]

    s = small.tile([128, NCHUNK], F32)     # squared norms (row 128c+p at [p, c])
    ls = small.tile([128, NCHUNK], F32)
    r = small.tile([128, NCHUNK], F32)     # 1/norm
    r_dve = small.tile([128, NCHUNK], F32)  # DVE-local copy
    iprobe = small.tile([1, 1], BF16)      # DVE probe of ident (Pool->DVE edge)
    rs = small.tile([128, 32], F32)        # accum slots (m, g)
    rt = small.tile([128, 8], F32)         # row totals per Mtile
    lg = small.tile([128, 8], F32)
    logsum = small.tile([128, 1], F32)
    possum = small.tile([128, 1], F32)
    fin = small.tile([128, 1], F32)
    fin2 = small.tile([128, 1], F32)       # ACT-written copy (matmul 1-wait rule)
    ones = small.tile([128, 1], F32)       # ACT-written
    ident = small.tile([128, 128], BF16)
    fin_sb = small.tile([1, 1], F32)
    pos_scr = small.tile([128, RPC], BF16)
    negE2 = small.tile([128, 1], F32)

    nc.vector.memset(negE2, -E2)
    make_identity(nc, ident)
    # DVE probe-read of ident: every later DVE op now transitively implies
    # the identity is built, letting the strip pass drop Pool waits from
    # the transpose matmuls (which can carry only one sync wait).
    nc.vector.tensor_copy(iprobe, ident[0:1, 0:1])
    # ones written by ACT so the final matmul waits on ACT only
    nc.scalar.activation(out=ones, in_=negE2, func=AF.Copy, bias=1.0, scale=0.0)

    x_src = x_ap.rearrange("(c p) d -> p c d", p=128)

    # ---- prep: load + norms + normalize + transpose, pipelined per group ----
    with tc.tile_pool(name="prep_ps", bufs=4, space="PSUM") as prep_ps:
        pts = []
        for g in range(8):
            sl = slice(8 * g, 8 * g + 8)
            nc.sync.dma_start(out=x_g[g][:, :, :], in_=x_src[:, sl, :])
            nc.vector.tensor_mul(
                xsq_g[g][:, :, :], x_g[g][:, :, :], x_g[g][:, :, :]
            )
            nc.vector.tensor_reduce(
                out=s[:, sl],
                in_=xsq_g[g][:, :, :],
                axis=mybir.AxisListType.X,
                op=ALU.add,
            )
            # r = exp(-0.5*ln(s)) == s^-1/2 ; exp+ln share one ACT table set
            nc.scalar.activation(out=ls[:, sl], in_=s[:, sl], func=AF.Ln)
            nc.scalar.activation(
                out=r[:, sl], in_=ls[:, sl], func=AF.Exp, scale=-0.5
            )
            nc.vector.tensor_copy(r_dve[:, sl], r[:, sl])
            for c in range(8 * g, 8 * g + 8):
                nc.vector.tensor_scalar_mul(
                    out=xb[:, c, :],
                    in0=x_g[c // 8][:, c % 8, :],
                    scalar1=r_dve[:, c : c + 1],
                )
            if g % 2 == 1:
                tg = g // 2
                pt = prep_ps.tile([128, 2048], BF16, tag="pt", name=f"pt_{tg}")
                for k in range(16):
                    ch = 16 * tg + k
                    nc.tensor.transpose(
                        pt[:, 128 * k : 128 * (k + 1)], xb[:, ch, :], ident
                    )
                # copy on ACT: matmuls consuming xnT then wait on ACT only
                nc.scalar.copy(xnT[tg][:, :], pt[:, :])
                pts.append(pt)

    # ---- main loop: sim block rows [0,1024) x all columns ----
    # col-group-major: the 8 exps of group t depend only on transpose-copy t,
    # so ACT's FIFO never head-of-line blocks on later prep
    ps = ctx.enter_context(tc.tile_pool(name="ps", bufs=2, space="PSUM"))
    for g in range(4):
        for m in range(8):
            pm = ps.tile([128, 2048], F32, tag="pm", name=f"pm_{m}_{g}")
            lhsT = xnT[0][:, 128 * m : 128 * (m + 1)]
            for k in range(4):
                nc.tensor.matmul(
                    pm[:, 512 * k : 512 * (k + 1)],
                    lhsT=lhsT,
                    rhs=xnT[g][:, 512 * k : 512 * (k + 1)],
                    start=True,
                    stop=True,
                )
            e_t = esc.tile([128, 2048], BF16, tag="e", name=f"e_{m}_{g}")
            j = 8 * g + m
            nc.scalar.activation(
                out=e_t[:, :],
                in_=pm[:, :],
                func=AF.Exp,
                scale=2.0,
                accum_out=rs[:, j : j + 1],
            )

    # ---- positive-pair term: sum over my rows of sim(i, i+N) ----
    # local pos column of local row i is always i + 4096 (rotation invariant)
    nc.vector.tensor_mul(pos_scr, xnT[0][:, 0:RPC], xnT[2][:, 0:RPC])
    nc.vector.tensor_reduce(
        out=possum, in_=pos_scr, axis=mybir.AxisListType.X, op=ALU.add
    )

    # ---- finals ----
    # rs columns are g-major (col 8g+m); sum over g per m via a strided view
    nc.vector.tensor_reduce(
        out=rt,
        in_=rs.rearrange("p (g m) -> p m g", m=8),
        axis=mybir.AxisListType.X,
        op=ALU.add,
    )
    # lg = ln(rowtotal - e^2), logsum = sum over the 8 Mtiles
    nc.scalar.activation(
        out=lg, in_=rt, func=AF.Ln, bias=negE2[:, :], scale=1.0, accum_out=logsum
    )
    # fin = logsum - 2 * possum
    nc.vector.scalar_tensor_tensor(
        out=fin,
        in0=possum,
        scalar=-2.0,
        in1=logsum,
        op0=ALU.mult,
        op1=ALU.add,
    )
    nc.scalar.copy(fin2, fin)  # ACT hop: final matmul waits on ACT only
    # partition reduce via ones-matmul
    pf = ps.tile([128, 2048], F32, tag="pm", name="pf")
    nc.tensor.matmul(
        pf[0:1, 0:1].bitcast(F32), lhsT=fin2, rhs=ones, start=True, stop=True
    )
    nc.vector.tensor_copy(fin_sb, pf[0:1, 0:1])
    # SWDGE for the tiny output write: the HWDGE direct-2D encoding only
    # carries one sync wait and this DMA needs a data wait on DVE
    nc.gpsimd.dma_start(out=out_ap, in_=fin_sb)


def _strip_self_waits(nc):
    """Drop engine-self semaphore waits from Matmult/Activation instructions.

    PE and ACT are strict in-order single queues whose semaphores increment
    at instruction completion in program order, so a wait on the engine's own
    semaphore is always transitively implied by queue order (and by the
    cross-engine wait that released the PSUM slot). Tile emits them anyway
    (its wait assignment is not transitively minimal across processors), and
    the Matmult instruction encoding only has room for ONE sync wait, so the
    extra self-wait breaks walrus codegen ("Too many sync wait commands").
    """
    eng_prefix = {
        mybir.EngineType.PE: "PE_",
        mybir.EngineType.Activation: "Activation_",
        mybir.EngineType.DVE: "DVE_",
    }
    for bb in nc.main_func.blocks:
        for ins in bb.instructions:
            si = ins.sync_info
            if si is None:
                continue
            if type(ins).__name__ == "InstDrain":
                # The tail drain waits on every engine + HWDGE queue sem,
                # overflowing its (<=4) wait slots. In this kernel the output
                # DMA's completion (DMASW0>=16) transitively implies all of
                # them: the SWDGE dma_start is the last Pool instruction and
                # waited on DVE's last instruction, which waited on PE's
                # last, which waited on ACT's last; the x-load DMAHW queue
                # waits are covered by the gpsimd squares. So a drain that
                # carries a DMASW wait needs only that wait.
                w = list(si.on_wait)
                if len(w) > 4 and any(
                    (x.ant_name or "").startswith("DMASW") for x in w
                ):
                    si.on_wait = [
                        x for x in w if (x.ant_name or "").startswith("DMASW")
                    ]
                continue
            pfx = eng_prefix.get(getattr(ins, "engine", None))
            if pfx is None:
                continue
            w = list(si.on_wait)
            w2 = [x for x in w if not (x.ant_name or "").startswith(pfx)]
            if type(ins).__name__ == "InstMatmult":
                # Pool only produces the identity matrix here, and the DVE
                # probe-read of it precedes every DVE-produced matmul input,
                # so any Pool wait on a matmul is transitively covered by
                # its DVE wait.
                w2 = [x for x in w2 if not (x.ant_name or "").startswith("Pool_")]
            if len(w2) != len(w):
                si.on_wait = w2


def _build(strip: bool = True):
    from contextlib import ExitStack

    nc = bass.Bass("TRN2", debug=False, num_devices=NCORES)
    x_in = nc.dram_tensor("x", [N2, D], F32, kind="ExternalInput")
    out = nc.dram_tensor("out", [1, 1], F32, kind="ExternalOutput")
    with tile.TileContext(nc) as tc:
        with ExitStack() as ctx:
            _emit(tc, ctx, out.ap(), x_in.ap())
    if strip:
        # CoreSim's race detector models engines as concurrent and would
        # flag the removed (redundant-on-HW) self-waits; validate numerics
        # with strip=False, ship with strip=True.
        _strip_self_waits(nc)
    return nc


_NC_CACHE = None


def _get_nc():
    global _NC_CACHE
    if _NC_CACHE is None:
        _NC_CACHE = _build()
    return _NC_CACHE


def kernel(**inputs) -> np.ndarray:
    x = np.ascontiguousarray(
        np.asarray(inputs["projected_vectors"]), dtype=np.float32
    )
    assert x.shape == (N2, D)
    nc = _get_nc()
    in_maps = [
        {"x": np.ascontiguousarray(np.roll(x, -RPC * c, axis=0))}
        for c in range(NCORES)
    ]
    res = run_bass_kernel_spmd(nc, in_maps, core_ids=list(range(NCORES)))
    total = np.float32(0.0)
    for rmap in res.results:
        total += np.float32(rmap["out"][0, 0])
    return np.asarray(total, dtype=np.float32)


if __name__ == "__main__":
    xt = np.random.randn(N2, D).astype(np.float32)
    print(kernel(projected_vectors=xt))
